# revision 5
# baseline (speedup 1.0000x reference)
"""Distributed GCN (3x GCNConv+BN+PReLU -> MLP head -> argmax) on 8 TRN2 cores.

Sharding: nodes split across 8 cores (12500 each); edges partitioned by
destination core; per layer the (h @ W * dinv) table is AllGathered, then each
core dma_gathers its edges' source rows (by src-chunk, int16-indexed) and
scatter-adds them via one-hot matmuls into PSUM windows of 512 destination
nodes. BatchNorm statistics are AllReduced. All activations are kept
feature-major ([128 feat, nodes]) so BN/PReLU use per-partition ACT ops.
"""
import os
import numpy as np

import concourse.bass as bass
import concourse.tile as tile
import concourse.mybir as mybir
from concourse import bacc
from concourse.bass_utils import run_bass_kernel_spmd
from concourse.masks import make_identity

F32 = mybir.dt.float32
I16 = mybir.dt.int16
I32 = mybir.dt.int32
AL = mybir.AluOpType
AF = mybir.ActivationFunctionType

P = 128
BIG = 65536.0


class Cfg:
    def __init__(self, N=100000, E=600000, D=128, L=3, C=40, ncores=8,
                 nchunks=4, WIN=512, BGROUPS=8, EPS=1e-5):
        assert N % ncores == 0 and N % nchunks == 0
        self.N, self.E, self.D, self.L, self.C = N, E, D, L, C
        self.ncores, self.nchunks, self.WIN, self.BGROUPS, self.EPS = \
            ncores, nchunks, WIN, BGROUPS, EPS
        self.NSH = N // ncores
        self.CHUNK = N // nchunks
        assert self.CHUNK <= 32767, "chunk must be int16-addressable"
        self.NWIN = -(-self.NSH // WIN)
        self.NCHK = -(-self.NSH // P)

    def wlen(self, w):
        return min(self.WIN, self.NSH - w * self.WIN)


def _prep(edge_index, cfg):
    """Host graph preprocessing -> shared program structure + per-core data."""
    N, ncores, nchunks, WIN = cfg.N, cfg.ncores, cfg.nchunks, cfg.WIN
    NSH, CHUNK, NWIN = cfg.NSH, cfg.CHUNK, cfg.NWIN
    src = np.concatenate([edge_index[0], np.arange(N, dtype=np.int64)])
    dst = np.concatenate([edge_index[1], np.arange(N, dtype=np.int64)])
    deg = np.bincount(dst, minlength=N).astype(np.float32)
    dinv = (1.0 / np.sqrt(np.maximum(deg, 1.0))).astype(np.float32)

    # bucket edges per (core, chunk, window), sorted by local dst
    cell = {}  # (k, c, w) -> (es_rel, ed_rel, dst_glob)
    kk = dst // NSH
    for k in range(ncores):
        m = kk == k
        es, ed = src[m], dst[m] - k * NSH
        c_of = es // CHUNK
        w_of = ed // WIN
        for c in range(nchunks):
            for w in range(NWIN):
                mm = (c_of == c) & (w_of == w)
                e_s, e_d = es[mm], ed[mm]
                o = np.argsort(e_d, kind="stable")
                cell[(k, c, w)] = (e_s[o] - c * CHUNK, e_d[o] - w * WIN,
                                   e_d[o] + k * NSH)

    # uniform slot counts
    S = np.zeros((nchunks, NWIN), dtype=np.int64)
    for c in range(nchunks):
        for w in range(NWIN):
            mx = max(len(cell[(k, c, w)][0]) for k in range(ncores))
            S[c, w] = -(-mx // P) * P if mx > 0 else 0

    TOT = int(S.sum())
    NGRP = TOT // P
    # groups in (c, w, j) order; batches per chunk
    groups = []   # (c, w, j, gidx)
    batches = []  # (c, col0, nslots, grp0, ngrp)
    slot_cursor = 0
    gidx = 0
    for c in range(nchunks):
        cgrp0 = gidx
        for w in range(NWIN):
            for j in range(S[c, w] // P):
                groups.append((c, w, j == 0, j == S[c, w] // P - 1, gidx))
                gidx += 1
        # batches over this chunk's groups
        cgrps = gidx - cgrp0
        g = 0
        while g < cgrps:
            ng = min(cfg.BGROUPS, cgrps - g)
            batches.append((c, (slot_cursor + g * P) // 16, ng * P, cgrp0 + g, ng))
            g += ng
        slot_cursor += cgrps * P
    assert gidx == NGRP

    # per-core data arrays + union spans
    idx_flat = np.zeros((ncores, TOT), dtype=np.int16)
    dstrel = np.full((ncores, P, NGRP), -4096.0, dtype=np.float32)
    dinvd = np.zeros((ncores, P, NGRP), dtype=np.float32)
    j0 = np.full(NGRP, 10 ** 9, dtype=np.int64)
    j1 = np.zeros(NGRP, dtype=np.int64)
    base = 0
    g = 0
    for c in range(nchunks):
        for w in range(NWIN):
            ns = int(S[c, w])
            for k in range(ncores):
                e_s, e_d, d_g = cell[(k, c, w)]
                n = len(e_s)
                idx_flat[k, base:base + n] = e_s.astype(np.int16)
                for jj in range(ns // P):
                    lo, hi = jj * P, min((jj + 1) * P, n)
                    if hi > lo:
                        sl = slice(lo, hi)
                        dstrel[k, 0:hi - lo, g + jj] = e_d[sl]
                        dinvd[k, 0:hi - lo, g + jj] = dinv[d_g[sl]]
                        j0[g + jj] = min(j0[g + jj], int(e_d[sl].min()))
                        j1[g + jj] = max(j1[g + jj], int(e_d[sl].max()) + 1)
            base += ns
            g += ns // P
    j0 = np.minimum(j0, j1 - 1).clip(0)  # empty groups -> [0,1)
    j1 = np.maximum(j1, j0 + 1)

    # pack idxs: flat i -> [i % 16, i // 16], replicated to 128 partitions
    idx16 = np.zeros((ncores, P, TOT // 16), dtype=np.int16)
    for k in range(ncores):
        idx16[k] = np.tile(idx_flat[k].reshape(-1, 16).T, (8, 1))

    gspans = [(int(a), int(b)) for a, b in zip(j0, j1)]
    return dict(dinv=dinv, S=S, TOT=TOT, NGRP=NGRP, groups=groups,
                batches=batches, gspans=gspans, idx16=idx16,
                dstrel=dstrel, dinvd=dinvd)


def _build(cfg, meta, prelu_a):
    N, D, L, C = cfg.N, cfg.D, cfg.L, cfg.C
    NSH, CHUNK, WIN, NWIN, NCHK = cfg.NSH, cfg.CHUNK, cfg.WIN, cfg.NWIN, cfg.NCHK
    NGRP, TOT = meta["NGRP"], meta["TOT"]
    groups, batches, gspans = meta["groups"], meta["batches"], meta["gspans"]

    nc = bacc.Bacc("TRN2")
    xT_in = nc.dram_tensor("xT", [P, NSH], F32, kind="ExternalInput")
    W_in = [nc.dram_tensor(f"W{l}", [D, D], F32, kind="ExternalInput")
            for l in range(L)]
    Wh1_in = nc.dram_tensor("Wh1", [D, D], F32, kind="ExternalInput")
    Wh2_in = nc.dram_tensor("Wh2", [D, C], F32, kind="ExternalInput")
    gam_in = nc.dram_tensor("gam", [P, L], F32, kind="ExternalInput")
    bet_in = nc.dram_tensor("bet", [P, L], F32, kind="ExternalInput")
    bh1_in = nc.dram_tensor("bh1", [P, 1], F32, kind="ExternalInput")
    bh2_in = nc.dram_tensor("bh2r", [P, C], F32, kind="ExternalInput")
    dvo_in = nc.dram_tensor("dinv_own", [P, NCHK], F32, kind="ExternalInput")
    idx_in = nc.dram_tensor("idx16", [P, TOT // 16], I16, kind="ExternalInput")
    dsr_in = nc.dram_tensor("dstrel", [P, NGRP], F32, kind="ExternalInput")
    dvd_in = nc.dram_tensor("dinvd", [P, NGRP], F32, kind="ExternalInput")
    iota_in = nc.dram_tensor("iota", [P, WIN], F32, kind="ExternalInput")

    emb_out = nc.dram_tensor("emb", [NSH, D], F32, kind="ExternalOutput")
    log_out = nc.dram_tensor("logits", [NSH, C], F32, kind="ExternalOutput")
    out_out = nc.dram_tensor("outi", [NSH, 1], I32, kind="ExternalOutput")

    RG = [list(range(cfg.ncores))]

    with tile.TileContext(nc) as tc:
        with tc.tile_pool(name="pers", bufs=1) as pers, \
             tc.tile_pool(name="gb", bufs=6) as gb, \
             tc.tile_pool(name="pp", bufs=4) as pp, \
             tc.tile_pool(name="tmp", bufs=2) as tmp, \
             tc.tile_pool(name="sm", bufs=2) as sm, \
             tc.tile_pool(name="wps", bufs=3, space="PSUM") as wps, \
             tc.tile_pool(name="tps", bufs=2, space="PSUM") as tps, \
             tc.tile_pool(name="dram", bufs=1, space="DRAM") as dram:

            # --- load persistent inputs ---
            hT = pers.tile([P, NSH], F32, tag="hT")
            hacc = pers.tile([P, NSH], F32, tag="hacc")
            nc.sync.dma_start(hT[:], xT_in[:])
            w_sb = []
            for l in range(L):
                wt = pers.tile([D, D], F32, tag=f"W{l}", name=f"Wsb{l}")
                nc.sync.dma_start(wt[:], W_in[l][:])
                w_sb.append(wt)
            wh1 = pers.tile([D, D], F32, tag="wh1")
            wh2 = pers.tile([D, C], F32, tag="wh2")
            gam = pers.tile([P, L], F32, tag="gam")
            bet = pers.tile([P, L], F32, tag="bet")
            bh1 = pers.tile([P, 1], F32, tag="bh1")
            bh2r = pers.tile([P, C], F32, tag="bh2r")
            dvo = pers.tile([P, NCHK], F32, tag="dvo")
            idx16 = pers.tile([P, TOT // 16], I16, tag="idx16")
            dsr = pers.tile([P, NGRP], F32, tag="dsr")
            dvd = pers.tile([P, NGRP], F32, tag="dvd")
            iota = pers.tile([P, WIN], F32, tag="iota")
            for t, s in [(wh1, Wh1_in), (wh2, Wh2_in), (gam, gam_in),
                         (bet, bet_in), (bh1, bh1_in), (bh2r, bh2_in),
                         (dvo, dvo_in), (idx16, idx_in), (dsr, dsr_in),
                         (dvd, dvd_in), (iota, iota_in)]:
                nc.sync.dma_start(t[:], s[:])
            ident = pers.tile([P, P], F32, tag="ident")
            make_identity(nc, ident[:])
            pzero = pers.tile([P, WIN], F32, tag="pzero")
            nc.gpsimd.memset(pzero[:], 0.0)

            for l in range(L):
                # --- table = (h @ W_l) * dinv, then AllGather ---
                ag_in = dram.tile([NSH, D], F32, tag=f"agin{l}", name=f"agin{l}")
                table = dram.tile([N, D], F32, tag=f"tab{l}", name=f"tab{l}",
                                  addr_space="Shared")
                for ch in range(NCHK):
                    n0 = ch * P
                    nn = min(P, NSH - n0)
                    ps = tps.tile([P, D], F32, tag="tmm", name="tmmps")
                    nc.tensor.matmul(ps[:nn, :], hT[:, n0:n0 + nn], w_sb[l][:],
                                     start=True, stop=True)
                    st = tmp.tile([P, D], F32, tag="tstage", name="tstage")
                    nc.scalar.activation(st[:nn, :], ps[:nn, :], AF.Copy,
                                         scale=dvo[:nn, ch:ch + 1])
                    nc.sync.dma_start(ag_in[n0:n0 + nn, :], st[:nn, :])
                nc.gpsimd.collective_compute(
                    "AllGather", AL.bypass, replica_groups=RG,
                    ins=[ag_in[:].opt()], outs=[table[:].opt()])

                # --- gather + one-hot scatter ---
                seen = set()
                cur_ps = None
                for (c, col0, nslots, grp0, ngrp) in batches:
                    gbt = gb.tile([P, nslots // P, D], F32, tag="gbuf",
                                  name="gbuf")
                    nc.gpsimd.dma_gather(
                        gbt[:], table[c * CHUNK:(c + 1) * CHUNK, :],
                        idx16[:, col0:col0 + nslots // 16],
                        nslots, nslots, D)
                    for gl in range(ngrp):
                        c_, w, first, last, gidx = groups[grp0 + gl]
                        assert c_ == c
                        a, b = gspans[gidx]
                        Pt = pp.tile([P, WIN], F32, tag="P", name="Pt")
                        nc.vector.tensor_scalar(
                            Pt[:, a:b], iota[:, a:b], dsr[:, gidx:gidx + 1],
                            dvd[:, gidx:gidx + 1], AL.is_equal, AL.mult)
                        if first:
                            cur_ps = wps.tile([P, WIN], F32, tag="wps",
                                              name="wpsps")
                            wl0 = cfg.wlen(w)
                            # zero the whole bank uniformly so partial-span
                            # accumulating matmuls are well-defined
                            nc.tensor.matmul(cur_ps[:, :wl0], ident[:],
                                             pzero[:, :wl0], start=True,
                                             stop=False)
                        nc.tensor.matmul(cur_ps[:, a:b], gbt[:, gl, :],
                                         Pt[:, a:b], start=False, stop=last)
                        if last:
                            wl = cfg.wlen(w)
                            w0 = w * WIN
                            if w not in seen:
                                seen.add(w)
                                nc.vector.tensor_copy(hacc[:, w0:w0 + wl],
                                                      cur_ps[:, :wl])
                            else:
                                nc.vector.tensor_tensor(
                                    hacc[:, w0:w0 + wl], hacc[:, w0:w0 + wl],
                                    cur_ps[:, :wl], AL.add)
                assert len(seen) == NWIN, f"unseen windows {seen}"

                # --- BN stats (global via AllReduce) ---
                s1 = sm.tile([P, NWIN], F32, tag="s1", name="s1")
                s2 = sm.tile([P, NWIN], F32, tag="s2", name="s2")
                for w in range(NWIN):
                    wl = cfg.wlen(w)
                    w0 = w * WIN
                    scr = tmp.tile([P, WIN], F32, tag="scr", name="scr")
                    nc.scalar.activation(scr[:, :wl], hacc[:, w0:w0 + wl],
                                         AF.Identity, accum_out=s1[:, w:w + 1])
                    scr2 = tmp.tile([P, WIN], F32, tag="scr", name="scr2")
                    nc.scalar.activation(scr2[:, :wl], hacc[:, w0:w0 + wl],
                                         AF.Square, accum_out=s2[:, w:w + 1])
                S12 = sm.tile([P, 2], F32, tag="S12", name="S12")
                nc.vector.tensor_reduce(S12[:, 0:1], s1[:, :NWIN],
                                        mybir.AxisListType.X, AL.add)
                nc.vector.tensor_reduce(S12[:, 1:2], s2[:, :NWIN],
                                        mybir.AxisListType.X, AL.add)
                ar_in = dram.tile([P, 2], F32, tag=f"arin{l}", name=f"arin{l}")
                ar_out = dram.tile([P, 2], F32, tag=f"arout{l}",
                                   name=f"arout{l}", addr_space="Shared")
                nc.sync.dma_start(ar_in[:], S12[:])
                nc.gpsimd.collective_compute(
                    "AllReduce", AL.add, replica_groups=RG,
                    ins=[ar_in[:].opt()], outs=[ar_out[:].opt()])
                SR = sm.tile([P, 2], F32, tag="SR", name="SR")
                nc.sync.dma_start(SR[:], ar_out[:])

                # mu = S1/N ; var = S2/N - mu^2 ; s = gamma/sqrt(var+eps)
                # b = beta - mu*s   (conv bias cancels in BN)
                mu = sm.tile([P, 1], F32, tag="mu", name="mu")
                nc.vector.tensor_scalar(mu[:], SR[:, 0:1], 1.0 / N, None, AL.mult)
                ex2 = sm.tile([P, 1], F32, tag="ex2", name="ex2")
                nc.vector.tensor_scalar(ex2[:], SR[:, 1:2], 1.0 / N, None, AL.mult)
                mu2 = sm.tile([P, 1], F32, tag="mu2", name="mu2")
                nc.vector.tensor_tensor(mu2[:], mu[:], mu[:], AL.mult)
                var = sm.tile([P, 1], F32, tag="var", name="var")
                nc.vector.tensor_tensor(var[:], ex2[:], mu2[:], AL.subtract)
                vpe = sm.tile([P, 1], F32, tag="vpe", name="vpe")
                nc.vector.tensor_scalar(vpe[:], var[:], cfg.EPS, None, AL.add)
                sd = sm.tile([P, 1], F32, tag="sd", name="sd")
                nc.scalar.activation(sd[:], vpe[:], AF.Sqrt)
                rs = sm.tile([P, 1], F32, tag="rs", name="rs")
                nc.vector.reciprocal(rs[:], sd[:])
                s_col = sm.tile([P, 1], F32, tag="s_col", name="s_col")
                nc.vector.tensor_tensor(s_col[:], rs[:], gam[:, l:l + 1], AL.mult)
                mus = sm.tile([P, 1], F32, tag="mus", name="mus")
                nc.vector.tensor_tensor(mus[:], mu[:], s_col[:], AL.mult)
                b_col = sm.tile([P, 1], F32, tag="b_col", name="b_col")
                nc.vector.tensor_tensor(b_col[:], bet[:, l:l + 1], mus[:],
                                        AL.subtract)

                # --- BN affine + PReLU -> hT (next layer input) ---
                al_ = float(prelu_a[l])
                for w in range(NWIN):
                    wl = cfg.wlen(w)
                    w0 = w * WIN
                    bn = tmp.tile([P, WIN], F32, tag="bn", name="bn")
                    nc.scalar.activation(bn[:, :wl], hacc[:, w0:w0 + wl],
                                         AF.Identity, bias=b_col[:, 0:1],
                                         scale=s_col[:, 0:1])
                    nc.vector.tensor_scalar(hT[:, w0:w0 + wl], bn[:, :wl],
                                            0.0, None, AL.max)
                    r2 = tmp.tile([P, WIN], F32, tag="r2", name="r2")
                    nc.vector.tensor_scalar(r2[:, :wl], bn[:, :wl],
                                            0.0, al_, AL.min, AL.mult)
                    nc.vector.tensor_tensor(hT[:, w0:w0 + wl],
                                            hT[:, w0:w0 + wl], r2[:, :wl],
                                            AL.add)

            # --- head: emb / logits / argmax ---
            ib = pers.tile([P, C], F32, tag="ib")
            nc.vector.tensor_scalar(ib[:], iota[:, :C], BIG, None, AL.subtract)

            for ch in range(NCHK):
                n0 = ch * P
                nn = min(P, NSH - n0)
                tp = tps.tile([P, D], F32, tag="tmm", name="trps")
                nc.tensor.transpose(tp[:nn, :], hT[:, n0:n0 + nn], ident[:])
                st = tmp.tile([P, D], F32, tag="tstage", name="embst")
                nc.scalar.activation(st[:nn, :], tp[:nn, :], AF.Copy)
                nc.sync.dma_start(emb_out[n0:n0 + nn, :], st[:nn, :])

            for w in range(NWIN):
                wl = cfg.wlen(w)
                w0 = w * WIN
                t1ps = wps.tile([P, WIN], F32, tag="wps", name="t1ps")
                nc.tensor.matmul(t1ps[:, :wl], wh1[:], hT[:, w0:w0 + wl],
                                 start=True, stop=True)
                t1 = tmp.tile([P, WIN], F32, tag="t1", name="t1")
                nc.scalar.activation(t1[:, :wl], t1ps[:, :wl], AF.Relu,
                                     bias=bh1[:, 0:1])
                for i0 in range(0, wl, P):
                    nn = min(P, wl - i0)
                    gn0 = w0 + i0
                    lgps = tps.tile([P, D], F32, tag="tmm", name="lgps")
                    nc.tensor.matmul(lgps[:nn, :C], t1[:, i0:i0 + nn], wh2[:],
                                     start=True, stop=True)
                    lg = tmp.tile([P, C], F32, tag="lg", name="lg")
                    nc.vector.tensor_tensor(lg[:nn, :], lgps[:nn, :C],
                                            bh2r[:nn, :], AL.add)
                    nc.sync.dma_start(log_out[gn0:gn0 + nn, :], lg[:nn, :])
                    mx = sm.tile([P, 1], F32, tag="mx", name="mx")
                    nc.vector.tensor_reduce(mx[:nn, :], lg[:nn, :],
                                            mybir.AxisListType.X, AL.max)
                    eq = tmp.tile([P, C], F32, tag="eq", name="eq")
                    nc.vector.tensor_scalar(eq[:nn, :], lg[:nn, :],
                                            mx[:nn, 0:1], None, AL.is_equal)
                    vv = tmp.tile([P, C], F32, tag="vv", name="vv")
                    nc.vector.tensor_tensor(vv[:nn, :], eq[:nn, :],
                                            ib[:nn, :], AL.mult)
                    mn = sm.tile([P, 1], F32, tag="mn", name="mn")
                    nc.vector.tensor_reduce(mn[:nn, :], vv[:nn, :],
                                            mybir.AxisListType.X, AL.min)
                    mni = sm.tile([P, 1], F32, tag="mni", name="mni")
                    nc.vector.tensor_scalar(mni[:nn, :], mn[:nn, :], BIG,
                                            None, AL.add)
                    ami = sm.tile([P, 1], I32, tag="ami", name="ami")
                    nc.vector.tensor_copy(ami[:nn, :], mni[:nn, :])
                    nc.sync.dma_start(out_out[gn0:gn0 + nn, :], ami[:nn, :])

    nc.compile()
    return nc


def _in_maps(cfg, meta, x, Ws, Wh1, bh1, Wh2, bh2, gammas, betas):
    N, D, L, C, NSH, NCHK = cfg.N, cfg.D, cfg.L, cfg.C, cfg.NSH, cfg.NCHK
    dinv = meta["dinv"]
    iota = np.tile(np.arange(cfg.WIN, dtype=np.float32)[None, :], (P, 1))
    maps = []
    for k in range(cfg.ncores):
        sh = slice(k * NSH, (k + 1) * NSH)
        dvo = np.zeros((P, NCHK), np.float32)
        dv = dinv[sh]
        for ch in range(NCHK):
            n0 = ch * P
            nn = min(P, NSH - n0)
            dvo[:nn, ch] = dv[n0:n0 + nn]
        m = dict(
            xT=np.ascontiguousarray(x[sh].T),
            Wh1=Wh1, Wh2=Wh2,
            gam=np.ascontiguousarray(gammas.T),
            bet=np.ascontiguousarray(betas.T),
            bh1=bh1.reshape(D, 1) if bh1.size == D else bh1.reshape(-1, 1),
            bh2r=np.tile(bh2[None, :], (P, 1)),
            dinv_own=dvo,
            idx16=meta["idx16"][k],
            dstrel=meta["dstrel"][k],
            dinvd=meta["dinvd"][k],
            iota=iota,
        )
        for l in range(L):
            m[f"W{l}"] = Ws[l]
        maps.append(m)
    return maps


_cache = {}


def kernel(x, edge_index, Ws, bs, gammas, betas, prelu_a, Wh1, bh1, Wh2, bh2):
    x = np.asarray(x, np.float32)
    edge_index = np.asarray(edge_index)
    Ws, gammas, betas = (np.asarray(a, np.float32) for a in (Ws, gammas, betas))
    prelu_a = np.asarray(prelu_a, np.float32)
    Wh1, bh1 = np.asarray(Wh1, np.float32), np.asarray(bh1, np.float32)
    Wh2, bh2 = np.asarray(Wh2, np.float32), np.asarray(bh2, np.float32)

    cfg = Cfg(N=x.shape[0], E=edge_index.shape[1], D=x.shape[1],
              L=Ws.shape[0], C=Wh2.shape[1])
    key = ("k", cfg.N, cfg.E, edge_index.tobytes()[:64],
           int(edge_index.sum()), tuple(np.asarray(prelu_a, np.float64)))
    if key not in _cache:
        meta = _prep(edge_index.astype(np.int64), cfg)
        nc = _build(cfg, meta, prelu_a)
        _cache.clear()
        _cache[key] = (meta, nc)
    meta, nc = _cache[key]

    maps = _in_maps(cfg, meta, x, Ws, Wh1, bh1, Wh2, bh2, gammas, betas)
    trace = bool(int(os.environ.get("KERNEL_TRACE", "0")))
    if trace:
        try:
            import prof_shim
            prof_shim.install()
        except Exception:
            pass
    res = run_bass_kernel_spmd(nc, maps, core_ids=list(range(cfg.ncores)),
                               trace=trace)
    kernel.last_exec_time_ns = res.exec_time_ns

    emb = np.concatenate([res.results[k]["emb"] for k in range(cfg.ncores)], 0)
    logits = np.concatenate([res.results[k]["logits"]
                             for k in range(cfg.ncores)], 0)
    outi = np.concatenate([res.results[k]["outi"]
                           for k in range(cfg.ncores)], 0).reshape(-1)
    return emb, logits, outi.astype(np.int32)


# revision 22
# speedup vs baseline: 2.0031x; 2.0031x over previous
"""Distributed GCN (3x GCNConv+BN+PReLU -> MLP head -> argmax) on 8 TRN2 cores.

Sharding: nodes split across 8 cores; edges partitioned by destination core.
Per layer the (h @ W * dinv) table is AllGathered in 4 pieces (so the Q7-bound
edge gather overlaps the collective); each core dma_gathers its edges' source
rows (int16-indexed within a piece) and scatter-adds them via matmuls against
host-precomputed one-hot-times-dinv P matrices into PSUM windows of 512
destination nodes. BatchNorm statistics are AllReduced. Activations stay
feature-major ([128 feat, nodes]) so BN/PReLU use per-partition ACT ops.
"""
import os
import numpy as np

import concourse.bass as bass
import concourse.tile as tile
import concourse.mybir as mybir
from concourse import bacc
from concourse.bass_utils import run_bass_kernel_spmd
from concourse.masks import make_identity

F32 = mybir.dt.float32
I16 = mybir.dt.int16
I32 = mybir.dt.int32
AL = mybir.AluOpType
AF = mybir.ActivationFunctionType

P = 128
BIG = 65536.0


class Cfg:
    def __init__(self, N=100000, E=600000, D=128, L=3, C=40, ncores=8,
                 npieces=4, WIN=512, BGROUPS=8, EPS=1e-5):
        assert N % ncores == 0
        self.N, self.E, self.D, self.L, self.C = N, E, D, L, C
        self.ncores, self.npieces, self.WIN, self.BGROUPS, self.EPS = \
            ncores, npieces, WIN, BGROUPS, EPS
        self.NSH = N // ncores
        self.NWIN = -(-self.NSH // WIN)
        self.NCHK = -(-self.NSH // P)
        # piece boundaries in shard-row space, multiples of 128
        base = (self.NSH // npieces) // P * P
        starts = [min(i * base, self.NSH) for i in range(npieces)] + [self.NSH]
        self.p_start = starts[:-1]
        self.p_len = [starts[i + 1] - starts[i] for i in range(npieces)]
        assert all(pl > 0 for pl in self.p_len)
        assert all(pl * ncores <= 32767 for pl in self.p_len), \
            "piece tables must be int16-addressable"

    def wlen(self, w):
        return min(self.WIN, self.NSH - w * self.WIN)


def _prep(edge_index, cfg):
    """Host preprocessing -> shared program structure + per-core data."""
    N, ncores, WIN = cfg.N, cfg.ncores, cfg.WIN
    NSH, NWIN, NP = cfg.NSH, cfg.NWIN, cfg.npieces
    src = np.concatenate([edge_index[0], np.arange(N, dtype=np.int64)])
    dst = np.concatenate([edge_index[1], np.arange(N, dtype=np.int64)])
    deg = np.bincount(dst, minlength=N).astype(np.float32)
    dinv = (1.0 / np.sqrt(np.maximum(deg, 1.0))).astype(np.float32)

    p_starts = np.array(cfg.p_start + [NSH], dtype=np.int64)

    # per (core, piece): edge lists sorted by local dst
    cell = {}
    kk = dst // NSH
    for k in range(ncores):
        m = kk == k
        es, ed = src[m], dst[m] - k * NSH
        r = es % NSH
        ksrc = es // NSH
        pc = np.searchsorted(p_starts, r, side="right") - 1
        idxp = ksrc * np.array(cfg.p_len)[pc] + (r - p_starts[pc])
        for p_ in range(NP):
            mm = pc == p_
            e_i, e_d = idxp[mm], ed[mm]
            o = np.argsort(e_d, kind="stable")
            cell[(k, p_)] = (e_i[o], e_d[o])

    S = [max(-(-len(cell[(k, p_)][0]) // P) * P for k in range(ncores))
         for p_ in range(NP)]
    TOT = int(sum(S))
    NGRP = TOT // P

    # pairs (group x window) with union spans across cores
    pairs = []        # (piece, grp_local, w, a, b, off, first_pw, last_pw)
    batches = []      # (piece, idxcol0, nslots, pair_lo, pair_hi, grp_lo, poff0)
    idx_flat = np.zeros((ncores, TOT), dtype=np.int16)
    sbase = 0
    off = 0
    for p_ in range(NP):
        ns = S[p_]
        ng = ns // P
        for k in range(ncores):
            e_i, _ = cell[(k, p_)]
            idx_flat[k, sbase:sbase + len(e_i)] = e_i.astype(np.int16)
        # window content per group per core
        piece_pairs = []
        for g in range(ng):
            lo, hi = g * P, (g + 1) * P
            wset = {}
            for k in range(ncores):
                _, e_d = cell[(k, p_)]
                dd = e_d[lo:min(hi, len(e_d))]
                if len(dd) == 0:
                    continue
                for w in np.unique(dd // WIN):
                    sel = dd[(dd // WIN) == w]
                    a, b = int(sel.min() - w * WIN), int(sel.max() - w * WIN) + 1
                    if w in wset:
                        wset[w] = (min(wset[w][0], a), max(wset[w][1], b))
                    else:
                        wset[w] = (a, b)
            for w in sorted(wset):
                a, b = wset[w]
                piece_pairs.append([p_, g, int(w), a, b, 0, False, False])
        # first/last per (piece, window)
        seen_first = set()
        last_of = {}
        for i, pr in enumerate(piece_pairs):
            w = pr[2]
            if w not in seen_first:
                seen_first.add(w)
                pr[6] = True
            last_of[w] = i
        for w, i in last_of.items():
            piece_pairs[i][7] = True
        # batches of up to BGROUPS groups; P loads in half-batch units so the
        # pcat staging tiles stay small. pcat offsets are pload-contiguous.
        pi = 0
        g = 0
        while g < ng:
            nb = min(cfg.BGROUPS, ng - g)
            lo_pair = pi
            blds = []
            for g2 in range(g, g + nb, max(1, cfg.BGROUPS // 2)):
                nb2 = min(max(1, cfg.BGROUPS // 2), g + nb - g2)
                poff0 = off
                lo2 = pi
                while pi < len(piece_pairs) and piece_pairs[pi][1] < g2 + nb2:
                    piece_pairs[pi][5] = off
                    off += piece_pairs[pi][4] - piece_pairs[pi][3]
                    pi += 1
                blds.append((len(pairs) + lo2, len(pairs) + pi, poff0,
                             off - poff0))
            batches.append((p_, sbase // 16 + g * P // 16, nb * P,
                            len(pairs) + lo_pair, len(pairs) + pi, g, blds))
            g += nb
        assert pi == len(piece_pairs)
        pairs.extend(tuple(pr) for pr in piece_pairs)
        sbase += ns
    SPTOT = off

    # per-core P content
    pcat = np.zeros((ncores, P, SPTOT), dtype=np.float32)
    for p_ in range(NP):
        ppairs = [pr for pr in pairs if pr[0] == p_]
        for k in range(ncores):
            _, e_d = cell[(k, p_)]
            dv = dinv  # dinv of global dst = dinv[k*NSH + ed]
            for (pp, g, w, a, b, o, _f, _l) in ppairs:
                lo, hi = g * P, min((g + 1) * P, len(e_d))
                if hi <= lo:
                    continue
                dd = e_d[lo:hi]
                sel = np.nonzero((dd >= w * WIN + a) & (dd < w * WIN + b))[0]
                for s in sel:
                    col = o + int(dd[s]) - w * WIN - a
                    pcat[k, lo % P + s, col] = dv[k * NSH + int(dd[s])]

    idx16 = np.zeros((ncores, P, TOT // 16), dtype=np.int16)
    for k in range(ncores):
        idx16[k] = np.tile(idx_flat[k].reshape(-1, 16).T, (8, 1))

    # max pcat extent over the P-load units
    MAXBSPAN = 1
    for (p_, c0, nsl, plo, phi, glo, blds) in batches:
        for (_l, _h, _o, sp) in blds:
            MAXBSPAN = max(MAXBSPAN, sp)

    # max simultaneously-open PSUM windows (sizes the wps pool)
    maxopen = 1
    nopen = 0
    for pr in pairs:
        if pr[6]:
            nopen += 1
            maxopen = max(maxopen, nopen)
        if pr[7]:
            nopen -= 1

    return dict(dinv=dinv, S=S, TOT=TOT, NGRP=NGRP, pairs=pairs,
                batches=batches, MAXBSPAN=MAXBSPAN,
                MAXOPEN=maxopen, SPTOT=SPTOT, idx16=idx16, pcat=pcat)


def _build(cfg, meta, prelu_a):
    N, D, L, C = cfg.N, cfg.D, cfg.L, cfg.C
    NSH, WIN, NWIN, NCHK, NP = cfg.NSH, cfg.WIN, cfg.NWIN, cfg.NCHK, cfg.npieces
    TOT, SPTOT = meta["TOT"], meta["SPTOT"]
    pairs, batches = meta["pairs"], meta["batches"]
    MAXBSPAN = meta["MAXBSPAN"]

    nc = bacc.Bacc("TRN2")
    xT_in = nc.dram_tensor("xT", [P, NSH], F32, kind="ExternalInput")
    W_in = [nc.dram_tensor(f"W{l}", [D, D], F32, kind="ExternalInput")
            for l in range(L)]
    Wh1_in = nc.dram_tensor("Wh1", [D, D], F32, kind="ExternalInput")
    Wh2_in = nc.dram_tensor("Wh2", [D, C], F32, kind="ExternalInput")
    gam_in = nc.dram_tensor("gam", [P, L], F32, kind="ExternalInput")
    bet_in = nc.dram_tensor("bet", [P, L], F32, kind="ExternalInput")
    bh1_in = nc.dram_tensor("bh1", [P, 1], F32, kind="ExternalInput")
    bh2_in = nc.dram_tensor("bh2r", [P, C], F32, kind="ExternalInput")
    dvo_in = nc.dram_tensor("dinv_own", [P, NCHK], F32, kind="ExternalInput")
    idx_in = nc.dram_tensor("idx16", [P, TOT // 16], I16, kind="ExternalInput")
    pcat_in = nc.dram_tensor("pcat", [P, SPTOT], F32, kind="ExternalInput")
    iota_in = nc.dram_tensor("iota", [P, max(C, 64)], F32, kind="ExternalInput")

    emb_out = nc.dram_tensor("emb", [NSH, D], F32, kind="ExternalOutput")
    log_out = nc.dram_tensor("logits", [NSH, C], F32, kind="ExternalOutput")
    out_out = nc.dram_tensor("outi", [P, NCHK], I32, kind="ExternalOutput")

    RG = [list(range(cfg.ncores))]

    with tile.TileContext(nc) as tc:
        with tc.tile_pool(name="pers", bufs=1) as pers, \
             tc.tile_pool(name="gb", bufs=5) as gb, \
             tc.tile_pool(name="pp", bufs=3) as pp, \
             tc.tile_pool(name="tmp", bufs=2) as tmp, \
             tc.tile_pool(name="sm", bufs=2) as sm, \
             tc.tile_pool(name="wps", bufs=min(6, max(3, meta["MAXOPEN"] + 1)),
                          space="PSUM") as wps, \
             tc.tile_pool(name="tps", bufs=2, space="PSUM") as tps, \
             tc.tile_pool(name="dram", bufs=1, space="DRAM") as dram:

            hT = pers.tile([P, NSH], F32, tag="hT")
            hacc = pers.tile([P, NSH], F32, tag="hacc")
            nc.sync.dma_start(hT[:], xT_in[:])
            w_sb = []
            for l in range(L):
                wt = pers.tile([D, D], F32, tag=f"W{l}", name=f"Wsb{l}")
                nc.sync.dma_start(wt[:], W_in[l][:])
                w_sb.append(wt)
            wh1 = pers.tile([D, D], F32, tag="wh1")
            wh2 = pers.tile([D, C], F32, tag="wh2")
            gam = pers.tile([P, L], F32, tag="gam")
            bet = pers.tile([P, L], F32, tag="bet")
            bh1 = pers.tile([P, 1], F32, tag="bh1")
            bh2r = pers.tile([P, C], F32, tag="bh2r")
            dvo = pers.tile([P, NCHK], F32, tag="dvo")
            idx16 = pers.tile([P, TOT // 16], I16, tag="idx16")
            iota = pers.tile([P, max(C, 64)], F32, tag="iota")
            for t, s in [(wh1, Wh1_in), (wh2, Wh2_in), (gam, gam_in),
                         (bet, bet_in), (bh1, bh1_in), (bh2r, bh2_in),
                         (dvo, dvo_in), (idx16, idx_in), (iota, iota_in)]:
                nc.sync.dma_start(t[:], s[:])
            ident = pers.tile([P, P], F32, tag="ident")
            make_identity(nc, ident[:])
            pzero = pers.tile([P, WIN], F32, tag="pzero")
            nc.gpsimd.memset(pzero[:], 0.0)

            for l in range(L):
                # --- table pieces = (h @ W_l) * dinv; 4 split AllGathers ---
                ag_in = [dram.tile([cfg.p_len[p_], D], F32,
                                   tag=f"agin{l}_{p_}", name=f"agin{l}_{p_}")
                         for p_ in range(NP)]
                tables = [dram.tile([cfg.p_len[p_] * cfg.ncores, D], F32,
                                    tag=f"tab{l}_{p_}", name=f"tab{l}_{p_}",
                                    addr_space="Shared")
                          for p_ in range(NP)]
                for p_ in range(NP):
                    r0 = cfg.p_start[p_]
                    for ch0 in range(0, cfg.p_len[p_], P):
                        n0 = r0 + ch0
                        nn = min(P, NSH - n0, cfg.p_len[p_] - ch0)
                        ps = tps.tile([P, D], F32, tag="tmm", name="tmmps")
                        nc.tensor.matmul(ps[:nn, :], hT[:, n0:n0 + nn],
                                         w_sb[l][:], start=True, stop=True)
                        st = tmp.tile([P, D], F32, tag="tstage", name="tstage")
                        nc.scalar.activation(st[:nn, :], ps[:nn, :], AF.Copy,
                                             scale=dvo[:nn, n0 // P:n0 // P + 1])
                        nc.sync.dma_start(ag_in[p_][ch0:ch0 + nn, :],
                                          st[:nn, :])
                    nc.gpsimd.collective_compute(
                        "AllGather", AL.bypass, replica_groups=RG,
                        ins=[ag_in[p_][:].opt()], outs=[tables[p_][:].opt()])

                # --- gather + scatter ---
                seen = set()
                open_ps = {}
                for (p_, c0, nsl, plo, phi, glo, blds) in batches:
                    gbt = gb.tile([P, nsl // P, D], F32, tag="gbuf",
                                  name="gbuf")
                    nc.gpsimd.dma_gather(
                        gbt[:], tables[p_][:],
                        idx16[:, c0:c0 + nsl // 16], nsl, nsl, D)
                    for (l2, h2, poff0, sp) in blds:
                        if sp > 0:
                            pb = pp.tile([P, MAXBSPAN], F32, tag="P",
                                         name="Pb")
                            nc.sync.dma_start(pb[:, :sp],
                                              pcat_in[:, poff0:poff0 + sp])
                        for i in range(l2, h2):
                            (pp_, g, w, a, b, o, first, last) = pairs[i]
                            wl = cfg.wlen(w)
                            if first:
                                cur = wps.tile([P, WIN], F32, tag="wps",
                                               name="wpsps")
                                open_ps[w] = cur
                                nc.tensor.matmul(cur[:, :wl], ident[:],
                                                 pzero[:, :wl], start=True,
                                                 stop=False)
                            cur = open_ps[w]
                            orel = o - poff0
                            nc.tensor.matmul(cur[:, a:b], gbt[:, g - glo, :],
                                             pb[:, orel:orel + (b - a)],
                                             start=False, stop=last)
                            if last:
                                w0 = w * WIN
                                if w not in seen:
                                    seen.add(w)
                                    nc.vector.tensor_copy(
                                        hacc[:, w0:w0 + wl], cur[:, :wl])
                                else:
                                    nc.vector.tensor_tensor(
                                        hacc[:, w0:w0 + wl],
                                        hacc[:, w0:w0 + wl],
                                        cur[:, :wl], AL.add)
                                del open_ps[w]
                assert len(seen) == NWIN, f"unseen windows {sorted(seen)}"

                # --- BN stats (global via AllReduce) ---
                s1 = sm.tile([P, NWIN], F32, tag="s1", name="s1")
                s2 = sm.tile([P, NWIN], F32, tag="s2", name="s2")
                for w in range(NWIN):
                    wl = cfg.wlen(w)
                    w0 = w * WIN
                    scr = tmp.tile([P, WIN], F32, tag="scr", name="scr",
                                   bufs=1)
                    nc.scalar.activation(scr[:, :wl], hacc[:, w0:w0 + wl],
                                         AF.Identity, accum_out=s1[:, w:w + 1])
                    scr2 = tmp.tile([P, WIN], F32, tag="scr", name="scr2",
                                    bufs=1)
                    nc.scalar.activation(scr2[:, :wl], hacc[:, w0:w0 + wl],
                                         AF.Square, accum_out=s2[:, w:w + 1])
                S12 = sm.tile([P, 2], F32, tag="S12", name="S12")
                nc.vector.tensor_reduce(S12[:, 0:1], s1[:, :NWIN],
                                        mybir.AxisListType.X, AL.add)
                nc.vector.tensor_reduce(S12[:, 1:2], s2[:, :NWIN],
                                        mybir.AxisListType.X, AL.add)
                ar_in = dram.tile([P, 2], F32, tag=f"arin{l}", name=f"arin{l}")
                ar_out = dram.tile([P, 2], F32, tag=f"arout{l}",
                                   name=f"arout{l}", addr_space="Shared")
                nc.sync.dma_start(ar_in[:], S12[:])
                nc.gpsimd.collective_compute(
                    "AllReduce", AL.add, replica_groups=RG,
                    ins=[ar_in[:].opt()], outs=[ar_out[:].opt()])
                SR = sm.tile([P, 2], F32, tag="SR", name="SR")
                nc.sync.dma_start(SR[:], ar_out[:])

                mu = sm.tile([P, 1], F32, tag="mu", name="mu")
                nc.vector.tensor_scalar(mu[:], SR[:, 0:1], 1.0 / N, None, AL.mult)
                ex2 = sm.tile([P, 1], F32, tag="ex2", name="ex2")
                nc.vector.tensor_scalar(ex2[:], SR[:, 1:2], 1.0 / N, None, AL.mult)
                mu2 = sm.tile([P, 1], F32, tag="mu2", name="mu2")
                nc.vector.tensor_tensor(mu2[:], mu[:], mu[:], AL.mult)
                var = sm.tile([P, 1], F32, tag="var", name="var")
                nc.vector.tensor_tensor(var[:], ex2[:], mu2[:], AL.subtract)
                vpe = sm.tile([P, 1], F32, tag="vpe", name="vpe")
                nc.vector.tensor_scalar(vpe[:], var[:], cfg.EPS, None, AL.add)
                sd = sm.tile([P, 1], F32, tag="sd", name="sd")
                nc.scalar.activation(sd[:], vpe[:], AF.Sqrt)
                rs = sm.tile([P, 1], F32, tag="rs", name="rs")
                nc.vector.reciprocal(rs[:], sd[:])
                s_col = sm.tile([P, 1], F32, tag="s_col", name="s_col")
                nc.vector.tensor_tensor(s_col[:], rs[:], gam[:, l:l + 1], AL.mult)
                mus = sm.tile([P, 1], F32, tag="mus", name="mus")
                nc.vector.tensor_tensor(mus[:], mu[:], s_col[:], AL.mult)
                b_col = sm.tile([P, 1], F32, tag="b_col", name="b_col")
                nc.vector.tensor_tensor(b_col[:], bet[:, l:l + 1], mus[:],
                                        AL.subtract)

                al_ = float(prelu_a[l])
                for w in range(NWIN):
                    wl = cfg.wlen(w)
                    w0 = w * WIN
                    bn = tmp.tile([P, WIN], F32, tag="bn", name="bn")
                    nc.scalar.activation(bn[:, :wl], hacc[:, w0:w0 + wl],
                                         AF.Identity, bias=b_col[:, 0:1],
                                         scale=s_col[:, 0:1])
                    nc.vector.tensor_scalar(hT[:, w0:w0 + wl], bn[:, :wl],
                                            0.0, None, AL.max)
                    r2 = tmp.tile([P, WIN], F32, tag="r2", name="r2")
                    nc.vector.tensor_scalar(r2[:, :wl], bn[:, :wl],
                                            0.0, al_, AL.min, AL.mult)
                    nc.vector.tensor_tensor(hT[:, w0:w0 + wl],
                                            hT[:, w0:w0 + wl], r2[:, :wl],
                                            AL.add)

            # --- head: emb / logits / argmax ---
            ib = pers.tile([P, 1, C], F32, tag="ib")
            nc.vector.tensor_scalar(ib[:].rearrange("p o c -> p (o c)"),
                                    iota[:, :C], BIG, None, AL.subtract)

            for ch in range(NCHK):
                n0 = ch * P
                nn = min(P, NSH - n0)
                tp = tps.tile([P, D], F32, tag="tmm", name="trps")
                nc.tensor.transpose(tp[:nn, :], hT[:, n0:n0 + nn], ident[:])
                st = tmp.tile([P, D], F32, tag="tstage", name="embst")
                nc.scalar.activation(st[:nn, :], tp[:nn, :], AF.Copy)
                nc.sync.dma_start(emb_out[n0:n0 + nn, :], st[:nn, :])

            ami_all = pers.tile([P, NCHK], I32, tag="ami")
            for w in range(NWIN):
                wl = cfg.wlen(w)
                w0 = w * WIN
                ni = -(-wl // P)
                t1ps = wps.tile([P, WIN], F32, tag="wps", name="t1ps")
                nc.tensor.matmul(t1ps[:, :wl], wh1[:], hT[:, w0:w0 + wl],
                                 start=True, stop=True)
                t1 = tmp.tile([P, WIN], F32, tag="t1", name="t1")
                nc.scalar.activation(t1[:, :wl], t1ps[:, :wl], AF.Relu,
                                     bias=bh1[:, 0:1])
                lgw = tmp.tile([P, 4 * C], F32, tag="lgw", name="lgw")
                for i in range(ni):
                    i0 = i * P
                    nn = min(P, wl - i0)
                    gn0 = w0 + i0
                    lgps = tps.tile([P, D], F32, tag="tmm", name="lgps")
                    nc.tensor.matmul(lgps[:nn, :C], t1[:, i0:i0 + nn], wh2[:],
                                     start=True, stop=True)
                    if nn < P:
                        # keep pad partitions finite for the batched argmax
                        nc.vector.tensor_copy(lgw[:, i * C:(i + 1) * C],
                                              bh2r[:])
                    nc.vector.tensor_tensor(lgw[:nn, i * C:(i + 1) * C],
                                            lgps[:nn, :C], bh2r[:nn, :],
                                            AL.add)
                    nc.sync.dma_start(log_out[gn0:gn0 + nn, :],
                                      lgw[:nn, i * C:(i + 1) * C])
                lg3 = lgw[:, :ni * C].rearrange("p (i c) -> p i c", c=C)
                mx = sm.tile([P, 4, 1], F32, tag="mx", name="mx")
                nc.vector.tensor_reduce(mx[:, :ni, :], lg3,
                                        mybir.AxisListType.X, AL.max)
                eq = tmp.tile([P, 4 * C], F32, tag="eq", name="eq")
                eq3 = eq[:, :ni * C].rearrange("p (i c) -> p i c", c=C)
                nc.vector.tensor_tensor(eq3, lg3,
                                        mx[:, :ni, :].to_broadcast([P, ni, C]),
                                        AL.is_equal)
                vv = tmp.tile([P, 4 * C], F32, tag="vv", name="vv")
                vv3 = vv[:, :ni * C].rearrange("p (i c) -> p i c", c=C)
                nc.vector.tensor_tensor(vv3, eq3,
                                        ib[:].to_broadcast([P, ni, C]),
                                        AL.mult)
                mn = sm.tile([P, 4, 1], F32, tag="mn", name="mn")
                nc.vector.tensor_reduce(mn[:, :ni, :], vv3,
                                        mybir.AxisListType.X, AL.min)
                mni = sm.tile([P, 4], F32, tag="mni", name="mni")
                nc.vector.tensor_scalar(
                    mni[:, :ni],
                    mn[:, :ni, :].rearrange("p i o -> p (i o)"), BIG,
                    None, AL.add)
                ch0 = w0 // P
                nc.vector.tensor_copy(ami_all[:, ch0:ch0 + ni], mni[:, :ni])
            nc.sync.dma_start(out_out[:], ami_all[:])

    nc.compile()
    return nc


def _in_maps(cfg, meta, x, Ws, Wh1, bh1, Wh2, bh2, gammas, betas):
    N, D, L, C, NSH, NCHK = cfg.N, cfg.D, cfg.L, cfg.C, cfg.NSH, cfg.NCHK
    dinv = meta["dinv"]
    iota = np.tile(np.arange(max(C, 64), dtype=np.float32)[None, :], (P, 1))
    maps = []
    for k in range(cfg.ncores):
        sh = slice(k * NSH, (k + 1) * NSH)
        dvo = np.zeros((P, NCHK), np.float32)
        dv = dinv[sh]
        for ch in range(NCHK):
            n0 = ch * P
            nn = min(P, NSH - n0)
            dvo[:nn, ch] = dv[n0:n0 + nn]
        m = dict(
            xT=np.ascontiguousarray(x[sh].T),
            Wh1=Wh1, Wh2=Wh2,
            gam=np.ascontiguousarray(gammas.T),
            bet=np.ascontiguousarray(betas.T),
            bh1=bh1.reshape(-1, 1),
            bh2r=np.tile(bh2[None, :], (P, 1)),
            dinv_own=dvo,
            idx16=meta["idx16"][k],
            pcat=meta["pcat"][k],
            iota=iota,
        )
        for l in range(L):
            m[f"W{l}"] = Ws[l]
        maps.append(m)
    return maps


_cache = {}


def kernel(x, edge_index, Ws, bs, gammas, betas, prelu_a, Wh1, bh1, Wh2, bh2):
    x = np.asarray(x, np.float32)
    edge_index = np.asarray(edge_index)
    Ws, gammas, betas = (np.asarray(a, np.float32) for a in (Ws, gammas, betas))
    prelu_a = np.asarray(prelu_a, np.float32)
    Wh1, bh1 = np.asarray(Wh1, np.float32), np.asarray(bh1, np.float32)
    Wh2, bh2 = np.asarray(Wh2, np.float32), np.asarray(bh2, np.float32)

    cfg = Cfg(N=x.shape[0], E=edge_index.shape[1], D=x.shape[1],
              L=Ws.shape[0], C=Wh2.shape[1])
    key = ("k2", cfg.N, cfg.E, edge_index.tobytes()[:64],
           int(edge_index.sum()), tuple(np.asarray(prelu_a, np.float64)))
    if key not in _cache:
        meta = _prep(edge_index.astype(np.int64), cfg)
        nc = _build(cfg, meta, prelu_a)
        _cache.clear()
        _cache[key] = (meta, nc)
    meta, nc = _cache[key]

    maps = _in_maps(cfg, meta, x, Ws, Wh1, bh1, Wh2, bh2, gammas, betas)
    trace = bool(int(os.environ.get("KERNEL_TRACE", "0")))
    if trace:
        try:
            import prof_shim
            prof_shim.install()
        except Exception:
            pass
    res = run_bass_kernel_spmd(nc, maps, core_ids=list(range(cfg.ncores)),
                               trace=trace)
    kernel.last_exec_time_ns = res.exec_time_ns

    emb = np.concatenate([res.results[k]["emb"] for k in range(cfg.ncores)], 0)
    logits = np.concatenate([res.results[k]["logits"]
                             for k in range(cfg.ncores)], 0)
    outs = []
    for k in range(cfg.ncores):
        oi = res.results[k]["outi"]  # [128, NCHK]
        outs.append(oi.T.reshape(-1)[:cfg.NSH])
    outi = np.concatenate(outs, 0)
    return emb, logits, outi.astype(np.int32)


# revision 24
# speedup vs baseline: 2.0057x; 1.0013x over previous
"""Distributed GCN (3x GCNConv+BN+PReLU -> MLP head -> argmax) on 8 TRN2 cores.

Sharding: nodes split across 8 cores; edges partitioned by destination core.
Per layer the (h @ W * dinv) table is AllGathered in 4 pieces (so the Q7-bound
edge gather overlaps the collective); each core dma_gathers its edges' source
rows (int16-indexed within a piece) and scatter-adds them via matmuls against
host-precomputed one-hot-times-dinv P matrices into PSUM windows of 512
destination nodes. BatchNorm statistics are AllReduced. Activations stay
feature-major ([128 feat, nodes]) so BN/PReLU use per-partition ACT ops.
"""
import os
import numpy as np

import concourse.bass as bass
import concourse.tile as tile
import concourse.mybir as mybir
from concourse import bacc
from concourse.bass_utils import run_bass_kernel_spmd
from concourse.masks import make_identity

F32 = mybir.dt.float32
I16 = mybir.dt.int16
I32 = mybir.dt.int32
AL = mybir.AluOpType
AF = mybir.ActivationFunctionType

P = 128
BIG = 65536.0


class Cfg:
    def __init__(self, N=100000, E=600000, D=128, L=3, C=40, ncores=8,
                 npieces=4, WIN=512, BGROUPS=8, EPS=1e-5):
        assert N % ncores == 0
        self.N, self.E, self.D, self.L, self.C = N, E, D, L, C
        self.ncores, self.npieces, self.WIN, self.BGROUPS, self.EPS = \
            ncores, npieces, WIN, BGROUPS, EPS
        self.NSH = N // ncores
        self.NWIN = -(-self.NSH // WIN)
        self.NCHK = -(-self.NSH // P)
        # piece boundaries in shard-row space, multiples of 128
        base = (self.NSH // npieces) // P * P
        starts = [min(i * base, self.NSH) for i in range(npieces)] + [self.NSH]
        self.p_start = starts[:-1]
        self.p_len = [starts[i + 1] - starts[i] for i in range(npieces)]
        assert all(pl > 0 for pl in self.p_len)
        assert all(pl * ncores <= 32767 for pl in self.p_len), \
            "piece tables must be int16-addressable"

    def wlen(self, w):
        return min(self.WIN, self.NSH - w * self.WIN)


def _prep(edge_index, cfg):
    """Host preprocessing -> shared program structure + per-core data."""
    N, ncores, WIN = cfg.N, cfg.ncores, cfg.WIN
    NSH, NWIN, NP = cfg.NSH, cfg.NWIN, cfg.npieces
    src = np.concatenate([edge_index[0], np.arange(N, dtype=np.int64)])
    dst = np.concatenate([edge_index[1], np.arange(N, dtype=np.int64)])
    deg = np.bincount(dst, minlength=N).astype(np.float32)
    dinv = (1.0 / np.sqrt(np.maximum(deg, 1.0))).astype(np.float32)

    p_starts = np.array(cfg.p_start + [NSH], dtype=np.int64)

    # per (core, piece): edge lists sorted by local dst
    cell = {}
    kk = dst // NSH
    for k in range(ncores):
        m = kk == k
        es, ed = src[m], dst[m] - k * NSH
        r = es % NSH
        ksrc = es // NSH
        pc = np.searchsorted(p_starts, r, side="right") - 1
        idxp = ksrc * np.array(cfg.p_len)[pc] + (r - p_starts[pc])
        for p_ in range(NP):
            mm = pc == p_
            e_i, e_d = idxp[mm], ed[mm]
            o = np.argsort(e_d, kind="stable")
            cell[(k, p_)] = (e_i[o], e_d[o])

    S = [max(-(-len(cell[(k, p_)][0]) // P) * P for k in range(ncores))
         for p_ in range(NP)]
    TOT = int(sum(S))
    NGRP = TOT // P

    # pairs (group x window) with union spans across cores
    pairs = []        # (piece, grp_local, w, a, b, off, first_pw, last_pw)
    batches = []      # (piece, idxcol0, nslots, pair_lo, pair_hi, grp_lo, poff0)
    idx_flat = np.zeros((ncores, TOT), dtype=np.int16)
    sbase = 0
    off = 0
    for p_ in range(NP):
        ns = S[p_]
        ng = ns // P
        for k in range(ncores):
            e_i, _ = cell[(k, p_)]
            idx_flat[k, sbase:sbase + len(e_i)] = e_i.astype(np.int16)
        # window content per group per core
        piece_pairs = []
        for g in range(ng):
            lo, hi = g * P, (g + 1) * P
            wset = {}
            for k in range(ncores):
                _, e_d = cell[(k, p_)]
                dd = e_d[lo:min(hi, len(e_d))]
                if len(dd) == 0:
                    continue
                for w in np.unique(dd // WIN):
                    sel = dd[(dd // WIN) == w]
                    a, b = int(sel.min() - w * WIN), int(sel.max() - w * WIN) + 1
                    if w in wset:
                        wset[w] = (min(wset[w][0], a), max(wset[w][1], b))
                    else:
                        wset[w] = (a, b)
            for w in sorted(wset):
                a, b = wset[w]
                piece_pairs.append([p_, g, int(w), a, b, 0, False, False])
        # first/last per (piece, window)
        seen_first = set()
        last_of = {}
        for i, pr in enumerate(piece_pairs):
            w = pr[2]
            if w not in seen_first:
                seen_first.add(w)
                pr[6] = True
            last_of[w] = i
        for w, i in last_of.items():
            piece_pairs[i][7] = True
        # batches of up to BGROUPS groups; P loads in half-batch units so the
        # pcat staging tiles stay small. pcat offsets are pload-contiguous.
        pi = 0
        g = 0
        while g < ng:
            nb = min(cfg.BGROUPS, ng - g)
            lo_pair = pi
            blds = []
            for g2 in range(g, g + nb, max(1, cfg.BGROUPS // 2)):
                nb2 = min(max(1, cfg.BGROUPS // 2), g + nb - g2)
                poff0 = off
                lo2 = pi
                while pi < len(piece_pairs) and piece_pairs[pi][1] < g2 + nb2:
                    piece_pairs[pi][5] = off
                    off += piece_pairs[pi][4] - piece_pairs[pi][3]
                    pi += 1
                blds.append((len(pairs) + lo2, len(pairs) + pi, poff0,
                             off - poff0))
            batches.append((p_, sbase // 16 + g * P // 16, nb * P,
                            len(pairs) + lo_pair, len(pairs) + pi, g, blds))
            g += nb
        assert pi == len(piece_pairs)
        pairs.extend(tuple(pr) for pr in piece_pairs)
        sbase += ns
    SPTOT = off

    # per-core P content
    pcat = np.zeros((ncores, P, SPTOT), dtype=np.float32)
    for p_ in range(NP):
        ppairs = [pr for pr in pairs if pr[0] == p_]
        for k in range(ncores):
            _, e_d = cell[(k, p_)]
            dv = dinv  # dinv of global dst = dinv[k*NSH + ed]
            for (pp, g, w, a, b, o, _f, _l) in ppairs:
                lo, hi = g * P, min((g + 1) * P, len(e_d))
                if hi <= lo:
                    continue
                dd = e_d[lo:hi]
                sel = np.nonzero((dd >= w * WIN + a) & (dd < w * WIN + b))[0]
                for s in sel:
                    col = o + int(dd[s]) - w * WIN - a
                    pcat[k, lo % P + s, col] = dv[k * NSH + int(dd[s])]

    idx16 = np.zeros((ncores, P, TOT // 16), dtype=np.int16)
    for k in range(ncores):
        idx16[k] = np.tile(idx_flat[k].reshape(-1, 16).T, (8, 1))

    # max pcat extent over the P-load units
    MAXBSPAN = 1
    for (p_, c0, nsl, plo, phi, glo, blds) in batches:
        for (_l, _h, _o, sp) in blds:
            MAXBSPAN = max(MAXBSPAN, sp)

    # max simultaneously-open PSUM windows (sizes the wps pool)
    maxopen = 1
    nopen = 0
    for pr in pairs:
        if pr[6]:
            nopen += 1
            maxopen = max(maxopen, nopen)
        if pr[7]:
            nopen -= 1

    return dict(dinv=dinv, S=S, TOT=TOT, NGRP=NGRP, pairs=pairs,
                batches=batches, MAXBSPAN=MAXBSPAN,
                MAXOPEN=maxopen, SPTOT=SPTOT, idx16=idx16, pcat=pcat)


def _build(cfg, meta, prelu_a):
    N, D, L, C = cfg.N, cfg.D, cfg.L, cfg.C
    NSH, WIN, NWIN, NCHK, NP = cfg.NSH, cfg.WIN, cfg.NWIN, cfg.NCHK, cfg.npieces
    TOT, SPTOT = meta["TOT"], meta["SPTOT"]
    pairs, batches = meta["pairs"], meta["batches"]
    MAXBSPAN = meta["MAXBSPAN"]

    nc = bacc.Bacc("TRN2")
    xT_in = nc.dram_tensor("xT", [P, NSH], F32, kind="ExternalInput")
    W_in = [nc.dram_tensor(f"W{l}", [D, D], F32, kind="ExternalInput")
            for l in range(L)]
    Wh1_in = nc.dram_tensor("Wh1", [D, D], F32, kind="ExternalInput")
    Wh2_in = nc.dram_tensor("Wh2", [D, C], F32, kind="ExternalInput")
    gam_in = nc.dram_tensor("gam", [P, L], F32, kind="ExternalInput")
    bet_in = nc.dram_tensor("bet", [P, L], F32, kind="ExternalInput")
    bh1_in = nc.dram_tensor("bh1", [P, 1], F32, kind="ExternalInput")
    bh2_in = nc.dram_tensor("bh2r", [P, C], F32, kind="ExternalInput")
    dvo_in = nc.dram_tensor("dinv_own", [P, NCHK], F32, kind="ExternalInput")
    idx_in = nc.dram_tensor("idx16", [P, TOT // 16], I16, kind="ExternalInput")
    pcat_in = nc.dram_tensor("pcat", [P, SPTOT], F32, kind="ExternalInput")
    iota_in = nc.dram_tensor("iota", [P, max(C, 64)], F32, kind="ExternalInput")

    emb_out = nc.dram_tensor("emb", [NSH, D], F32, kind="ExternalOutput")
    log_out = nc.dram_tensor("logits", [NSH, C], F32, kind="ExternalOutput")
    out_out = nc.dram_tensor("outi", [P, NCHK], I32, kind="ExternalOutput")

    RG = [list(range(cfg.ncores))]

    with tile.TileContext(nc) as tc:
        with tc.tile_pool(name="pers", bufs=1) as pers, \
             tc.tile_pool(name="gb", bufs=5) as gb, \
             tc.tile_pool(name="pp", bufs=3) as pp, \
             tc.tile_pool(name="tmp", bufs=2) as tmp, \
             tc.tile_pool(name="sm", bufs=2) as sm, \
             tc.tile_pool(name="wps", bufs=min(6, max(3, meta["MAXOPEN"] + 1)),
                          space="PSUM") as wps, \
             tc.tile_pool(name="tps", bufs=2, space="PSUM") as tps, \
             tc.tile_pool(name="dram", bufs=1, space="DRAM") as dram:

            hT = pers.tile([P, NSH], F32, tag="hT")
            hacc = pers.tile([P, NSH], F32, tag="hacc")
            nc.sync.dma_start(hT[:], xT_in[:])
            w_sb = []
            for l in range(L):
                wt = pers.tile([D, D], F32, tag=f"W{l}", name=f"Wsb{l}")
                nc.sync.dma_start(wt[:], W_in[l][:])
                w_sb.append(wt)
            wh1 = pers.tile([D, D], F32, tag="wh1")
            wh2 = pers.tile([D, C], F32, tag="wh2")
            gam = pers.tile([P, L], F32, tag="gam")
            bet = pers.tile([P, L], F32, tag="bet")
            bh1 = pers.tile([P, 1], F32, tag="bh1")
            bh2r = pers.tile([P, C], F32, tag="bh2r")
            dvo = pers.tile([P, NCHK], F32, tag="dvo")
            idx16 = pers.tile([P, TOT // 16], I16, tag="idx16")
            iota = pers.tile([P, max(C, 64)], F32, tag="iota")
            for t, s in [(wh1, Wh1_in), (wh2, Wh2_in), (gam, gam_in),
                         (bet, bet_in), (bh1, bh1_in), (bh2r, bh2_in),
                         (dvo, dvo_in), (idx16, idx_in), (iota, iota_in)]:
                nc.sync.dma_start(t[:], s[:])
            ident = pers.tile([P, P], F32, tag="ident")
            make_identity(nc, ident[:])
            pzero = pers.tile([P, WIN], F32, tag="pzero")
            nc.gpsimd.memset(pzero[:], 0.0)

            def emit_piece_table(l, p_, ag_in, tables):
                """matmul+scale the shard rows of piece p_ into ag_in, then
                AllGather them into tables[p_]."""
                r0 = cfg.p_start[p_]
                for ch0 in range(0, cfg.p_len[p_], P):
                    n0 = r0 + ch0
                    nn = min(P, NSH - n0, cfg.p_len[p_] - ch0)
                    ps = tps.tile([P, D], F32, tag="tmm", name="tmmps")
                    nc.tensor.matmul(ps[:nn, :], hT[:, n0:n0 + nn],
                                     w_sb[l][:], start=True, stop=True)
                    st = tmp.tile([P, D], F32, tag="tstage", name="tstage")
                    nc.scalar.activation(st[:nn, :], ps[:nn, :], AF.Copy,
                                         scale=dvo[:nn, n0 // P:n0 // P + 1])
                    nc.sync.dma_start(ag_in[p_][ch0:ch0 + nn, :], st[:nn, :])
                nc.gpsimd.collective_compute(
                    "AllGather", AL.bypass, replica_groups=RG,
                    ins=[ag_in[p_][:].opt()], outs=[tables[p_][:].opt()])

            def make_tables(l):
                ag_in = [dram.tile([cfg.p_len[p_], D], F32,
                                   tag=f"agin{l}_{p_}", name=f"agin{l}_{p_}")
                         for p_ in range(NP)]
                tables = [dram.tile([cfg.p_len[p_] * cfg.ncores, D], F32,
                                    tag=f"tab{l}_{p_}", name=f"tab{l}_{p_}",
                                    addr_space="Shared")
                          for p_ in range(NP)]
                return ag_in, tables

            ag_cur, tab_cur = make_tables(0)
            for p_ in range(NP):
                emit_piece_table(0, p_, ag_cur, tab_cur)

            for l in range(L):
                tables = tab_cur
                # --- gather + scatter ---
                seen = set()
                open_ps = {}
                for (p_, c0, nsl, plo, phi, glo, blds) in batches:
                    gbt = gb.tile([P, nsl // P, D], F32, tag="gbuf",
                                  name="gbuf")
                    nc.gpsimd.dma_gather(
                        gbt[:], tables[p_][:],
                        idx16[:, c0:c0 + nsl // 16], nsl, nsl, D)
                    for (l2, h2, poff0, sp) in blds:
                        if sp > 0:
                            pb = pp.tile([P, MAXBSPAN], F32, tag="P",
                                         name="Pb")
                            nc.sync.dma_start(pb[:, :sp],
                                              pcat_in[:, poff0:poff0 + sp])
                        for i in range(l2, h2):
                            (pp_, g, w, a, b, o, first, last) = pairs[i]
                            wl = cfg.wlen(w)
                            if first:
                                cur = wps.tile([P, WIN], F32, tag="wps",
                                               name="wpsps")
                                open_ps[w] = cur
                                nc.tensor.matmul(cur[:, :wl], ident[:],
                                                 pzero[:, :wl], start=True,
                                                 stop=False)
                            cur = open_ps[w]
                            orel = o - poff0
                            nc.tensor.matmul(cur[:, a:b], gbt[:, g - glo, :],
                                             pb[:, orel:orel + (b - a)],
                                             start=False, stop=last)
                            if last:
                                w0 = w * WIN
                                if w not in seen:
                                    seen.add(w)
                                    nc.vector.tensor_copy(
                                        hacc[:, w0:w0 + wl], cur[:, :wl])
                                else:
                                    nc.vector.tensor_tensor(
                                        hacc[:, w0:w0 + wl],
                                        hacc[:, w0:w0 + wl],
                                        cur[:, :wl], AL.add)
                                del open_ps[w]
                assert len(seen) == NWIN, f"unseen windows {sorted(seen)}"

                # --- BN stats (global via AllReduce) ---
                s1 = sm.tile([P, NWIN], F32, tag="s1", name="s1")
                s2 = sm.tile([P, NWIN], F32, tag="s2", name="s2")
                for w in range(NWIN):
                    wl = cfg.wlen(w)
                    w0 = w * WIN
                    scr = tmp.tile([P, WIN], F32, tag="scr", name="scr",
                                   bufs=1)
                    nc.scalar.activation(scr[:, :wl], hacc[:, w0:w0 + wl],
                                         AF.Identity, accum_out=s1[:, w:w + 1])
                    scr2 = tmp.tile([P, WIN], F32, tag="scr", name="scr2",
                                    bufs=1)
                    nc.scalar.activation(scr2[:, :wl], hacc[:, w0:w0 + wl],
                                         AF.Square, accum_out=s2[:, w:w + 1])
                S12 = sm.tile([P, 2], F32, tag="S12", name="S12")
                nc.vector.tensor_reduce(S12[:, 0:1], s1[:, :NWIN],
                                        mybir.AxisListType.X, AL.add)
                nc.vector.tensor_reduce(S12[:, 1:2], s2[:, :NWIN],
                                        mybir.AxisListType.X, AL.add)
                ar_in = dram.tile([P, 2], F32, tag=f"arin{l}", name=f"arin{l}")
                ar_out = dram.tile([P, 2], F32, tag=f"arout{l}",
                                   name=f"arout{l}", addr_space="Shared")
                nc.sync.dma_start(ar_in[:], S12[:])
                nc.gpsimd.collective_compute(
                    "AllReduce", AL.add, replica_groups=RG,
                    ins=[ar_in[:].opt()], outs=[ar_out[:].opt()])
                SR = sm.tile([P, 2], F32, tag="SR", name="SR")
                nc.sync.dma_start(SR[:], ar_out[:])

                mu = sm.tile([P, 1], F32, tag="mu", name="mu")
                nc.vector.tensor_scalar(mu[:], SR[:, 0:1], 1.0 / N, None, AL.mult)
                ex2 = sm.tile([P, 1], F32, tag="ex2", name="ex2")
                nc.vector.tensor_scalar(ex2[:], SR[:, 1:2], 1.0 / N, None, AL.mult)
                mu2 = sm.tile([P, 1], F32, tag="mu2", name="mu2")
                nc.vector.tensor_tensor(mu2[:], mu[:], mu[:], AL.mult)
                var = sm.tile([P, 1], F32, tag="var", name="var")
                nc.vector.tensor_tensor(var[:], ex2[:], mu2[:], AL.subtract)
                vpe = sm.tile([P, 1], F32, tag="vpe", name="vpe")
                nc.vector.tensor_scalar(vpe[:], var[:], cfg.EPS, None, AL.add)
                sd = sm.tile([P, 1], F32, tag="sd", name="sd")
                nc.scalar.activation(sd[:], vpe[:], AF.Sqrt)
                rs = sm.tile([P, 1], F32, tag="rs", name="rs")
                nc.vector.reciprocal(rs[:], sd[:])
                s_col = sm.tile([P, 1], F32, tag="s_col", name="s_col")
                nc.vector.tensor_tensor(s_col[:], rs[:], gam[:, l:l + 1], AL.mult)
                mus = sm.tile([P, 1], F32, tag="mus", name="mus")
                nc.vector.tensor_tensor(mus[:], mu[:], s_col[:], AL.mult)
                b_col = sm.tile([P, 1], F32, tag="b_col", name="b_col")
                nc.vector.tensor_tensor(b_col[:], bet[:, l:l + 1], mus[:],
                                        AL.subtract)

                al_ = float(prelu_a[l])

                def bn_apply(w):
                    wl = cfg.wlen(w)
                    w0 = w * WIN
                    bn = tmp.tile([P, WIN], F32, tag="bn", name="bn")
                    nc.scalar.activation(bn[:, :wl], hacc[:, w0:w0 + wl],
                                         AF.Identity, bias=b_col[:, 0:1],
                                         scale=s_col[:, 0:1])
                    nc.vector.tensor_scalar(hT[:, w0:w0 + wl], bn[:, :wl],
                                            0.0, None, AL.max)
                    r2 = tmp.tile([P, WIN], F32, tag="r2", name="r2")
                    nc.vector.tensor_scalar(r2[:, :wl], bn[:, :wl],
                                            0.0, al_, AL.min, AL.mult)
                    nc.vector.tensor_tensor(hT[:, w0:w0 + wl],
                                            hT[:, w0:w0 + wl], r2[:, :wl],
                                            AL.add)

                if l + 1 < L:
                    # interleave BN apply with the next layer's table pieces
                    # so the first AllGather fires early
                    ag_cur, tab_cur = make_tables(l + 1)
                    bn_done = 0
                    for p_ in range(NP):
                        w_hi = (cfg.p_start[p_] + cfg.p_len[p_] - 1) // WIN
                        while bn_done <= min(w_hi, NWIN - 1):
                            bn_apply(bn_done)
                            bn_done += 1
                        emit_piece_table(l + 1, p_, ag_cur, tab_cur)
                    while bn_done < NWIN:
                        bn_apply(bn_done)
                        bn_done += 1
                else:
                    for w in range(NWIN):
                        bn_apply(w)

            # --- head: emb / logits / argmax ---
            ib = pers.tile([P, 1, C], F32, tag="ib")
            nc.vector.tensor_scalar(ib[:].rearrange("p o c -> p (o c)"),
                                    iota[:, :C], BIG, None, AL.subtract)

            for ch in range(NCHK):
                n0 = ch * P
                nn = min(P, NSH - n0)
                tp = tps.tile([P, D], F32, tag="tmm", name="trps")
                nc.tensor.transpose(tp[:nn, :], hT[:, n0:n0 + nn], ident[:])
                st = tmp.tile([P, D], F32, tag="tstage", name="embst")
                nc.scalar.activation(st[:nn, :], tp[:nn, :], AF.Copy)
                nc.sync.dma_start(emb_out[n0:n0 + nn, :], st[:nn, :])

            ami_all = pers.tile([P, NCHK], I32, tag="ami")
            for w in range(NWIN):
                wl = cfg.wlen(w)
                w0 = w * WIN
                ni = -(-wl // P)
                t1ps = wps.tile([P, WIN], F32, tag="wps", name="t1ps")
                nc.tensor.matmul(t1ps[:, :wl], wh1[:], hT[:, w0:w0 + wl],
                                 start=True, stop=True)
                t1 = tmp.tile([P, WIN], F32, tag="t1", name="t1")
                nc.scalar.activation(t1[:, :wl], t1ps[:, :wl], AF.Relu,
                                     bias=bh1[:, 0:1])
                lgw = tmp.tile([P, 4 * C], F32, tag="lgw", name="lgw")
                for i in range(ni):
                    i0 = i * P
                    nn = min(P, wl - i0)
                    gn0 = w0 + i0
                    lgps = tps.tile([P, D], F32, tag="tmm", name="lgps")
                    nc.tensor.matmul(lgps[:nn, :C], t1[:, i0:i0 + nn], wh2[:],
                                     start=True, stop=True)
                    if nn < P:
                        # keep pad partitions finite for the batched argmax
                        nc.vector.tensor_copy(lgw[:, i * C:(i + 1) * C],
                                              bh2r[:])
                    nc.vector.tensor_tensor(lgw[:nn, i * C:(i + 1) * C],
                                            lgps[:nn, :C], bh2r[:nn, :],
                                            AL.add)
                    nc.sync.dma_start(log_out[gn0:gn0 + nn, :],
                                      lgw[:nn, i * C:(i + 1) * C])
                lg3 = lgw[:, :ni * C].rearrange("p (i c) -> p i c", c=C)
                mx = sm.tile([P, 4, 1], F32, tag="mx", name="mx")
                nc.vector.tensor_reduce(mx[:, :ni, :], lg3,
                                        mybir.AxisListType.X, AL.max)
                eq = tmp.tile([P, 4 * C], F32, tag="eq", name="eq")
                eq3 = eq[:, :ni * C].rearrange("p (i c) -> p i c", c=C)
                nc.vector.tensor_tensor(eq3, lg3,
                                        mx[:, :ni, :].to_broadcast([P, ni, C]),
                                        AL.is_equal)
                vv = tmp.tile([P, 4 * C], F32, tag="vv", name="vv")
                vv3 = vv[:, :ni * C].rearrange("p (i c) -> p i c", c=C)
                nc.vector.tensor_tensor(vv3, eq3,
                                        ib[:].to_broadcast([P, ni, C]),
                                        AL.mult)
                mn = sm.tile([P, 4, 1], F32, tag="mn", name="mn")
                nc.vector.tensor_reduce(mn[:, :ni, :], vv3,
                                        mybir.AxisListType.X, AL.min)
                mni = sm.tile([P, 4], F32, tag="mni", name="mni")
                nc.vector.tensor_scalar(
                    mni[:, :ni],
                    mn[:, :ni, :].rearrange("p i o -> p (i o)"), BIG,
                    None, AL.add)
                ch0 = w0 // P
                nc.vector.tensor_copy(ami_all[:, ch0:ch0 + ni], mni[:, :ni])
            nc.sync.dma_start(out_out[:], ami_all[:])

    nc.compile()
    return nc


def _in_maps(cfg, meta, x, Ws, Wh1, bh1, Wh2, bh2, gammas, betas):
    N, D, L, C, NSH, NCHK = cfg.N, cfg.D, cfg.L, cfg.C, cfg.NSH, cfg.NCHK
    dinv = meta["dinv"]
    iota = np.tile(np.arange(max(C, 64), dtype=np.float32)[None, :], (P, 1))
    maps = []
    for k in range(cfg.ncores):
        sh = slice(k * NSH, (k + 1) * NSH)
        dvo = np.zeros((P, NCHK), np.float32)
        dv = dinv[sh]
        for ch in range(NCHK):
            n0 = ch * P
            nn = min(P, NSH - n0)
            dvo[:nn, ch] = dv[n0:n0 + nn]
        m = dict(
            xT=np.ascontiguousarray(x[sh].T),
            Wh1=Wh1, Wh2=Wh2,
            gam=np.ascontiguousarray(gammas.T),
            bet=np.ascontiguousarray(betas.T),
            bh1=bh1.reshape(-1, 1),
            bh2r=np.tile(bh2[None, :], (P, 1)),
            dinv_own=dvo,
            idx16=meta["idx16"][k],
            pcat=meta["pcat"][k],
            iota=iota,
        )
        for l in range(L):
            m[f"W{l}"] = Ws[l]
        maps.append(m)
    return maps


_cache = {}


def kernel(x, edge_index, Ws, bs, gammas, betas, prelu_a, Wh1, bh1, Wh2, bh2):
    x = np.asarray(x, np.float32)
    edge_index = np.asarray(edge_index)
    Ws, gammas, betas = (np.asarray(a, np.float32) for a in (Ws, gammas, betas))
    prelu_a = np.asarray(prelu_a, np.float32)
    Wh1, bh1 = np.asarray(Wh1, np.float32), np.asarray(bh1, np.float32)
    Wh2, bh2 = np.asarray(Wh2, np.float32), np.asarray(bh2, np.float32)

    cfg = Cfg(N=x.shape[0], E=edge_index.shape[1], D=x.shape[1],
              L=Ws.shape[0], C=Wh2.shape[1])
    key = ("k2", cfg.N, cfg.E, edge_index.tobytes()[:64],
           int(edge_index.sum()), tuple(np.asarray(prelu_a, np.float64)))
    if key not in _cache:
        meta = _prep(edge_index.astype(np.int64), cfg)
        nc = _build(cfg, meta, prelu_a)
        _cache.clear()
        _cache[key] = (meta, nc)
    meta, nc = _cache[key]

    maps = _in_maps(cfg, meta, x, Ws, Wh1, bh1, Wh2, bh2, gammas, betas)
    trace = bool(int(os.environ.get("KERNEL_TRACE", "0")))
    if trace:
        try:
            import prof_shim
            prof_shim.install()
        except Exception:
            pass
    res = run_bass_kernel_spmd(nc, maps, core_ids=list(range(cfg.ncores)),
                               trace=trace)
    kernel.last_exec_time_ns = res.exec_time_ns

    emb = np.concatenate([res.results[k]["emb"] for k in range(cfg.ncores)], 0)
    logits = np.concatenate([res.results[k]["logits"]
                             for k in range(cfg.ncores)], 0)
    outs = []
    for k in range(cfg.ncores):
        oi = res.results[k]["outi"]  # [128, NCHK]
        outs.append(oi.T.reshape(-1)[:cfg.NSH])
    outi = np.concatenate(outs, 0)
    return emb, logits, outi.astype(np.int32)
